# revision 13
# baseline (speedup 1.0000x reference)
"""Bahdanau attention Trainium2 kernel.

B=32, S=2048, QH=KH=1024, H=512. Data-parallel over batch across 8 cores
(4 batches per core). Returns (context [B,1,H], weights [B,1,S]).

Per-core dataflow (per batch):
  keysT   : bf16 keys, DMA-xbar-transposed HBM->SBUF as [kh, s] tiles
  kproj   : PE bf16 matmul, UaT stationary; tanh+bias fused on ScalarE
            reading PSUM (bias column = q_proj[b] + Wa_b + Ua_b per h)
  scores  : PE fp32r matmul, Va column stationary over e.T tiles
  softmax : no max-subtraction (|scores| <~ 23 bounded by |Va|_1); Exp on
            ScalarE with fused accum_out denominator; normalize on DVE
  context : PE bf16 matmul, exp-weight columns stationary (PE-transposed
            from the exp row) over naturally-loaded bf16 keys; 1/den
            folded into the PSUM->SBUF copy
  Ya      : PE fp32r matmul, context columns stationary over YaT
Small weights are host-pre-transposed (layout only; all FLOPs on device).
"""

import sys
import types

import numpy as np
import ml_dtypes

B, S, KH, H = 32, 2048, 1024, 512
N_CORES = 8
B_CORE = B // N_CORES  # 4
BF16 = ml_dtypes.bfloat16

_CACHE = {}


def _install_ntff_hook():
    """Shim antenv.axon_hooks so run_bass_kernel_spmd(trace=True) works
    under axon. No-op if the real module exists."""
    try:
        import antenv.axon_hooks  # noqa: F401
        return
    except ImportError:
        pass
    try:
        from trn_agent_boot.trn_boot import _ntff_profile_via_ctypes
        hook = _ntff_profile_via_ctypes("/opt/axon/libaxon_pjrt.so")
    except Exception:
        hook = None
    m = types.ModuleType("antenv.axon_hooks")
    m.get_axon_ntff_profile_hook = lambda: hook
    sys.modules["antenv.axon_hooks"] = m


def _build_nc():
    import concourse.bacc as bacc
    import concourse.mybir as mybir
    import concourse.tile as tile
    from concourse.tile import add_dep_helper
    from concourse.masks import make_identity

    dt = mybir.dt
    F32, F32R, BF = dt.float32, dt.float32r, dt.bfloat16
    TANH = mybir.ActivationFunctionType.Tanh
    EXP = mybir.ActivationFunctionType.Exp
    COPY = mybir.ActivationFunctionType.Copy

    nc = bacc.Bacc(
        "TRN2", target_bir_lowering=False, debug=False, num_devices=N_CORES
    )

    # ---------------- DRAM I/O ----------------
    keys_bf = nc.dram_tensor("keys_bf16", [B_CORE, S, KH], BF, kind="ExternalInput").ap()
    keysT_bf = nc.dram_tensor("keysT_bf16", [B_CORE, KH, S], BF, kind="ExternalInput").ap()
    smalls = nc.dram_tensor("smalls", [128, 512], F32R, kind="ExternalInput").ap()
    WaT = nc.dram_tensor("WaT", [KH, H], F32R, kind="ExternalInput").ap()
    UaT_bf = nc.dram_tensor("UaT_bf16", [KH, H], BF, kind="ExternalInput").ap()
    YaT = nc.dram_tensor("YaT", [KH, H], F32R, kind="ExternalInput").ap()
    rows_pack = nc.dram_tensor("rows_pack", [1, 1024], F32R, kind="ExternalInput").ap()
    yab_row = nc.dram_tensor("yab_row", [1, H], F32R, kind="ExternalInput").ap()

    ctx_out = nc.dram_tensor("ctx_out", [B_CORE, H], F32, kind="ExternalOutput").ap()
    w_out = nc.dram_tensor("w_out", [B_CORE, S], F32, kind="ExternalOutput").ap()

    NKH = KH // 128  # 8 kh chunks
    NH = H // 128    # 4 h chunks
    NSB = S // 512   # 4 s blocks
    NSC = S // 128   # 16 s chunks

    with tile.TileContext(nc) as tc:
        with (
            tc.tile_pool(name="static", bufs=1) as st,
            tc.tile_pool(name="kT", bufs=2) as kT_pool,
            tc.tile_pool(name="knat", bufs=1) as knat_pool,
            tc.tile_pool(name="e", bufs=3) as e_pool,
            tc.tile_pool(name="rows", bufs=1) as row_pool,
            tc.tile_pool(name="small", bufs=2) as small_pool,
            tc.tile_pool(name="pe_ps", bufs=3, space="PSUM") as pe_ps,
            tc.tile_pool(name="sc_ps", bufs=2, space="PSUM") as sc_ps,
            tc.tile_pool(name="sm_ps", bufs=3, space="PSUM") as sm_ps,
        ):
            # ---------------- static loads ----------------
            ua_sb = st.tile([128, NKH * H], BF, tag="ua")      # 8KB/part
            wa_sb = st.tile([128, NKH * H], F32R, tag="wa")     # 16KB/part
            ya_sb = st.tile([128, NKH * H], F32R, tag="ya")     # 16KB/part
            for dst, srcw in ((ua_sb, UaT_bf), (wa_sb, WaT)):
                nc.sync.dma_start(
                    dst[:].rearrange("p (c h) -> p c h", c=NKH),
                    srcw.rearrange("(c p) h -> p c h", p=128),
                )
            smalls_sb = st.tile([128, 512], F32R, tag="smalls")
            nc.sync.dma_start(smalls_sb[:], smalls[:])
            rows_sb = st.tile([1, 1024], F32R, tag="rows")
            nc.sync.dma_start(rows_sb[:], rows_pack[:])
            id4 = st.tile([4, 4], F32, tag="id4")
            make_identity(nc, id4[:])
            one1 = st.tile([1, 1], F32, tag="one1")
            nc.gpsimd.memset(one1[:], 1.0)

            q_sb = smalls_sb                  # cols 0:32 -> q columns
            va_sb = smalls_sb[:, 32:36]       # cols 32:36 -> Va columns
            qb_sb = rows_sb[0:1, 0:H]         # qbias row
            va_b_ap = rows_sb[0:1, H:H + 1]   # Va_b scalar
            ones4 = rows_sb[0:1, H + 1:H + 5]  # [1,4] of ones
            one1r = rows_sb[0:1, H + 1:H + 2]  # [1,1] one (f32r)

            # ---------------- q projection (all 4 batches at once) ------
            # qproj[b, h] = sum_kh query[b,kh]*WaT[kh,h] + (Wa_b + Ua_b)[h]
            qp_ps = sm_ps.tile([4, H], F32, tag="sm")
            for c in range(NKH):
                nc.tensor.matmul(
                    qp_ps[:],
                    q_sb[:, c * B_CORE:(c + 1) * B_CORE],
                    wa_sb[:, c * H:(c + 1) * H],
                    start=(c == 0), stop=False,
                )
            nc.tensor.matmul(qp_ps[:], ones4, qb_sb, start=False, stop=True)
            # copy to SBUF row form, then transpose to columns [128, 4*NH]
            qp_row = st.tile([4, H], F32, tag="qprow")
            nc.scalar.copy(qp_row[:], qp_ps[:])
            qc_ps = sm_ps.tile([128, 4 * NH], F32, tag="sm")
            for hc in range(NH):
                nc.tensor.transpose(
                    qc_ps[:, hc * 4:(hc + 1) * 4],
                    qp_row[:, hc * 128:(hc + 1) * 128],
                    id4[:],
                )
            qcols = st.tile([128, 4 * NH], F32, tag="qcols")
            nc.scalar.copy(qcols[:], qc_ps[:])

            # ---------------- per batch ----------------
            for b in range(B_CORE):
                # -- keysT via DMA xbar transpose: 8 chunks [128, S] bf16
                kTt = kT_pool.tile([128, NKH * S], BF, tag="kT")
                for q in range(NSB):
                    nc.sync.dma_start(
                        kTt[:].rearrange("p (c s) -> p c s", c=NKH)[:, :, q * 512:(q + 1) * 512],
                        keysT_bf[b].rearrange("(c p) s -> p c s", p=128)[:, :, q * 512:(q + 1) * 512],
                    )

                exp_row = row_pool.tile([1, S], F32, tag="exp")
                den4 = small_pool.tile([1, 4], F32, tag="den")

                # -- kproj + tanh + scores + exp, streamed per s block.
                # The scores matmul trails its kproj group by one group so the
                # ScalarE tanh never stalls the PE.
                pending = []

                def emit_pending():
                    psb, phc, pet, ptile = pending.pop(0)
                    nc.tensor.matmul(
                        ptile[:], va_sb[:, phc:phc + 1], pet[:],
                        start=(phc == 0), stop=(phc == NH - 1),
                    )
                    if phc == NH - 1:
                        nc.scalar.activation(
                            exp_row[0:1, psb * 512:(psb + 1) * 512], ptile[:], EXP,
                            bias=va_b_ap,
                            accum_out=den4[0:1, psb:psb + 1],
                        )

                for sb in range(NSB):
                    sc_tile = sc_ps.tile([1, 512], F32, tag="sc")
                    for hc in range(NH):
                        eps = pe_ps.tile([128, 512], F32, tag="pe")
                        for c in range(NKH):
                            nc.tensor.matmul(
                                eps[:],
                                ua_sb[:, c * H + hc * 128: c * H + (hc + 1) * 128],
                                kTt[:, c * S + sb * 512: c * S + (sb + 1) * 512],
                                start=(c == 0), stop=(c == NKH - 1),
                            )
                        et = e_pool.tile([128, 512], F32R, tag="e")
                        tanh_ins = nc.scalar.activation(
                            et[:], eps[:], TANH,
                            bias=qcols[:, hc * 4 + b: hc * 4 + b + 1],
                        )
                        if sb == 0 and hc == 0:
                            first_tanh = tanh_ins
                        pending.append((sb, hc, et, sc_tile))
                        if len(pending) > 1:
                            emit_pending()
                # natural-layout keys for the context matmul: one fused DMA
                # (issued here so it doesn't compete with the kproj key loads)
                knt = knat_pool.tile([128, NSC * KH], BF, tag="knat")
                knat_dma = nc.gpsimd.dma_start(
                    knt[:].rearrange("p (sc j) -> p sc j", sc=NSC),
                    keys_bf[b].rearrange("(sc p) j -> p sc j", p=128),
                )
                add_dep_helper(first_tanh.ins, knat_dma.ins, sync=True,
                               reason="delay knat prefetch past batch start")

                while pending:
                    emit_pending()

                # -- denominator; normalize the exp row in place
                den1 = small_pool.tile([1, 1], F32, tag="den1")
                nc.vector.reduce_sum(den1[:], den4[:], axis=mybir.AxisListType.X)
                rcp = small_pool.tile([1, 1], F32, tag="rcp")
                nc.vector.reciprocal(rcp[:], den1[:])
                nc.vector.tensor_scalar_mul(exp_row[:], exp_row[:], rcp[:])
                nc.gpsimd.dma_start(w_out[b:b + 1, :], exp_row[:])

                # -- exp weight columns [128, 16] via PE transposes (unnormalized)
                wc_ps = sm_ps.tile([128, NSC], F32, tag="sm")
                for sc in range(NSC):
                    nc.tensor.transpose(
                        wc_ps[:, sc:sc + 1],
                        exp_row[0:1, sc * 128:(sc + 1) * 128],
                        one1[:],
                    )
                wcols = small_pool.tile([128, NSC], BF, tag="wcols")
                nc.scalar.copy(wcols[:], wc_ps[:])

                # -- context: two [1,512] psum halves over 16 s chunks
                ctx_ps = [sm_ps.tile([1, 512], F32, tag="sm", name=f"ctx_ps{i}")
                          for i in range(2)]
                for sc in range(NSC):
                    for half in range(2):
                        nc.tensor.matmul(
                            ctx_ps[half][:],
                            wcols[:, sc:sc + 1],
                            knt[:, sc * KH + half * 512: sc * KH + (half + 1) * 512],
                            start=(sc == 0), stop=(sc == NSC - 1),
                        )
                ctx_row = row_pool.tile([1, KH], F32, tag="ctxrow")
                for half in range(2):
                    nc.scalar.activation(
                        ctx_row[0:1, half * 512:(half + 1) * 512],
                        ctx_ps[half][:], COPY,
                    )

                if b == 0:
                    nc.gpsimd.dma_start(
                        ya_sb[:].rearrange("p (c h) -> p c h", c=NKH),
                        YaT.rearrange("(c p) h -> p c h", p=128),
                    )
                    yab_sb = st.tile([1, H], F32R, tag="yab")
                    nc.gpsimd.dma_start(yab_sb[:], yab_row[:])

                # -- context columns [128, 8] via PE transposes
                cT_ps = sm_ps.tile([128, NKH], F32, tag="sm")
                for c in range(NKH):
                    nc.tensor.transpose(
                        cT_ps[:, c:c + 1],
                        ctx_row[0:1, c * 128:(c + 1) * 128],
                        one1[:],
                    )
                cT = small_pool.tile([128, NKH], F32R, tag="cT")
                nc.scalar.copy(cT[:], cT_ps[:])

                # -- Ya projection: [1, 512] = sum_kh cT[:,c].T @ YaT chunk + Ya_b
                y_ps = sm_ps.tile([1, H], F32, tag="sm")
                for c in range(NKH):
                    nc.tensor.matmul(
                        y_ps[:], cT[:, c:c + 1], ya_sb[:, c * H:(c + 1) * H],
                        start=(c == 0), stop=False,
                    )
                nc.tensor.matmul(y_ps[:], one1r, yab_sb[:], start=False, stop=True)
                y_row = small_pool.tile([1, H], F32, tag="yrow")
                nc.scalar.copy(y_row[:], y_ps[:])
                nc.gpsimd.dma_start(ctx_out[b:b + 1, :], y_row[:])

    nc.compile()
    return nc


def _get_nc():
    if "nc" not in _CACHE:
        _install_ntff_hook()
        _CACHE["nc"] = _build_nc()
    return _CACHE["nc"]


def _prep_inputs(query, keys, Wa_w, Wa_b, Ua_w, Ua_b, Va_w, Va_b, Ya_w, Ya_b):
    """Host-side layout-only transforms + per-core sharding."""
    query = np.asarray(query, np.float32).reshape(B, KH)
    keys = np.asarray(keys, np.float32)
    keys_bf = keys.astype(BF16)  # [B, S, KH]

    WaT = np.ascontiguousarray(np.asarray(Wa_w, np.float32).T)          # [KH, H]
    UaT_bf = np.ascontiguousarray(np.asarray(Ua_w, np.float32).T).astype(BF16)
    YaT = np.ascontiguousarray(np.asarray(Ya_w, np.float32).T)          # [KH, H]
    yab_row = np.asarray(Ya_b, np.float32).reshape(1, H)
    rows_pack = np.zeros((1, 1024), np.float32)
    rows_pack[0, :H] = (np.asarray(Wa_b, np.float32) + np.asarray(Ua_b, np.float32))
    rows_pack[0, H] = np.float32(np.asarray(Va_b).reshape(-1)[0])
    rows_pack[0, H + 1:H + 5] = 1.0
    va_cols = np.ascontiguousarray(
        np.asarray(Va_w, np.float32).reshape(4, 128).T)                 # [128, 4]

    in_maps = []
    for i in range(N_CORES):
        bs = slice(i * B_CORE, (i + 1) * B_CORE)
        # smalls[:, 0:32]: q columns (q[p, c*4+b] = query[b, c*128+p]);
        # smalls[:, 32:36]: Va columns; rest zero-padded for fast DMA rows.
        smalls = np.zeros((128, 512), np.float32)
        smalls[:, 0:8 * B_CORE] = (
            query[bs].reshape(B_CORE, 8, 128).transpose(2, 1, 0).reshape(128, 8 * B_CORE)
        )
        smalls[:, 32:36] = va_cols
        in_maps.append({
            "keys_bf16": np.ascontiguousarray(keys_bf[bs]),
            "keysT_bf16": np.ascontiguousarray(keys_bf[bs].transpose(0, 2, 1)),
            "smalls": smalls,
            "WaT": WaT,
            "UaT_bf16": UaT_bf,
            "YaT": YaT,
            "rows_pack": rows_pack,
            "yab_row": yab_row,
        })
    return in_maps


def kernel(query, keys, Wa_w, Wa_b, Ua_w, Ua_b, Va_w, Va_b, Ya_w, Ya_b,
           _trace=False):
    from concourse.bass_utils import run_bass_kernel_spmd

    nc = _get_nc()
    in_maps = _prep_inputs(query, keys, Wa_w, Wa_b, Ua_w, Ua_b,
                           Va_w, Va_b, Ya_w, Ya_b)
    res = run_bass_kernel_spmd(
        nc, in_maps, core_ids=list(range(N_CORES)), trace=_trace
    )
    context = np.concatenate(
        [r["ctx_out"].reshape(B_CORE, 1, H) for r in res.results], axis=0
    ).astype(np.float32)
    weights = np.concatenate(
        [r["w_out"].reshape(B_CORE, 1, S) for r in res.results], axis=0
    ).astype(np.float32)
    if _trace:
        kernel._last_result = res
    return context, weights


# revision 14
# speedup vs baseline: 1.2708x; 1.2708x over previous
"""Bahdanau attention Trainium2 kernel.

B=32, S=2048, QH=KH=1024, H=512. Data-parallel over batch across 8 cores
(4 batches per core). Returns (context [B,1,H], weights [B,1,S]).

Per-core dataflow (per batch):
  keysT   : bf16 keys, DMA-xbar-transposed HBM->SBUF as [kh, s] tiles
  kproj   : PE bf16 matmul, UaT stationary; tanh+bias fused on ScalarE
            reading PSUM (bias column = q_proj[b] + Wa_b + Ua_b per h)
  scores  : PE fp32r matmul, Va column stationary over e.T tiles
  softmax : no max-subtraction (|scores| <~ 23 bounded by |Va|_1); Exp on
            ScalarE with fused accum_out denominator; normalize on DVE
  context : PE bf16 matmul, exp-weight columns stationary (PE-transposed
            from the exp row) over naturally-loaded bf16 keys; 1/den
            folded into the PSUM->SBUF copy
  Ya      : PE fp32r matmul, context columns stationary over YaT
Small weights are host-pre-transposed (layout only; all FLOPs on device).
"""

import sys
import types

import numpy as np
import ml_dtypes

B, S, KH, H = 32, 2048, 1024, 512
N_CORES = 8
B_CORE = B // N_CORES  # 4
BF16 = ml_dtypes.bfloat16

_CACHE = {}


def _install_ntff_hook():
    """Shim antenv.axon_hooks so run_bass_kernel_spmd(trace=True) works
    under axon. No-op if the real module exists."""
    try:
        import antenv.axon_hooks  # noqa: F401
        return
    except ImportError:
        pass
    try:
        from trn_agent_boot.trn_boot import _ntff_profile_via_ctypes
        hook = _ntff_profile_via_ctypes("/opt/axon/libaxon_pjrt.so")
    except Exception:
        hook = None
    m = types.ModuleType("antenv.axon_hooks")
    m.get_axon_ntff_profile_hook = lambda: hook
    sys.modules["antenv.axon_hooks"] = m


def _build_nc():
    import concourse.bacc as bacc
    import concourse.mybir as mybir
    import concourse.tile as tile
    from concourse.tile import add_dep_helper
    from concourse.masks import make_identity

    dt = mybir.dt
    F32, F32R, BF = dt.float32, dt.float32r, dt.bfloat16
    TANH = mybir.ActivationFunctionType.Tanh
    EXP = mybir.ActivationFunctionType.Exp
    COPY = mybir.ActivationFunctionType.Copy

    nc = bacc.Bacc(
        "TRN2", target_bir_lowering=False, debug=False, num_devices=N_CORES
    )

    # ---------------- DRAM I/O ----------------
    keys_bf = nc.dram_tensor("keys_bf16", [B_CORE, S, KH], BF, kind="ExternalInput").ap()
    keysT_bf = nc.dram_tensor("keysT_bf16", [B_CORE, KH, S], BF, kind="ExternalInput").ap()
    smalls = nc.dram_tensor("smalls", [128, 512], F32R, kind="ExternalInput").ap()
    WaT = nc.dram_tensor("WaT", [KH, H], F32R, kind="ExternalInput").ap()
    UaT_bf = nc.dram_tensor("UaT_bf16", [KH, H], BF, kind="ExternalInput").ap()
    YaT = nc.dram_tensor("YaT", [KH, H], F32R, kind="ExternalInput").ap()
    rows_pack = nc.dram_tensor("rows_pack", [1, 1024], F32R, kind="ExternalInput").ap()
    yab_row = nc.dram_tensor("yab_row", [1, H], F32R, kind="ExternalInput").ap()

    ctx_out = nc.dram_tensor("ctx_out", [B_CORE, H], F32, kind="ExternalOutput").ap()
    w_out = nc.dram_tensor("w_out", [B_CORE, S], F32, kind="ExternalOutput").ap()

    NKH = KH // 128  # 8 kh chunks
    NH = H // 128    # 4 h chunks
    NSB = S // 512   # 4 s blocks
    NSC = S // 128   # 16 s chunks

    with tile.TileContext(nc) as tc:
        with (
            tc.tile_pool(name="static", bufs=1) as st,
            tc.tile_pool(name="kT", bufs=2) as kT_pool,
            tc.tile_pool(name="knat", bufs=1) as knat_pool,
            tc.tile_pool(name="e", bufs=3) as e_pool,
            tc.tile_pool(name="rows", bufs=1) as row_pool,
            tc.tile_pool(name="small", bufs=2) as small_pool,
            tc.tile_pool(name="pe_ps", bufs=3, space="PSUM") as pe_ps,
            tc.tile_pool(name="sc_ps", bufs=2, space="PSUM") as sc_ps,
            tc.tile_pool(name="sm_ps", bufs=3, space="PSUM") as sm_ps,
        ):
            # ---------------- static loads ----------------
            ua_sb = st.tile([128, NKH * H], BF, tag="ua")      # 8KB/part
            wa_sb = st.tile([128, NKH * H], F32R, tag="wa")     # 16KB/part
            ya_sb = st.tile([128, NKH * H], F32R, tag="ya")     # 16KB/part
            for dst, srcw in ((ua_sb, UaT_bf), (wa_sb, WaT)):
                nc.sync.dma_start(
                    dst[:].rearrange("p (c h) -> p c h", c=NKH),
                    srcw.rearrange("(c p) h -> p c h", p=128),
                )
            smalls_sb = st.tile([128, 512], F32R, tag="smalls")
            nc.sync.dma_start(smalls_sb[:], smalls[:])
            rows_sb = st.tile([1, 1024], F32R, tag="rows")
            nc.sync.dma_start(rows_sb[:], rows_pack[:])
            id4 = st.tile([4, 4], F32, tag="id4")
            make_identity(nc, id4[:])
            one1 = st.tile([1, 1], F32, tag="one1")
            nc.gpsimd.memset(one1[:], 1.0)

            q_sb = smalls_sb                  # cols 0:32 -> q columns
            va_sb = smalls_sb[:, 32:36]       # cols 32:36 -> Va columns
            qb_sb = rows_sb[0:1, 0:H]         # qbias row
            va_b_ap = rows_sb[0:1, H:H + 1]   # Va_b scalar
            ones4 = rows_sb[0:1, H + 1:H + 5]  # [1,4] of ones
            one1r = rows_sb[0:1, H + 1:H + 2]  # [1,1] one (f32r)

            # ---------------- q projection (all 4 batches at once) ------
            # qproj[b, h] = sum_kh query[b,kh]*WaT[kh,h] + (Wa_b + Ua_b)[h]
            qp_ps = sm_ps.tile([4, H], F32, tag="sm")
            for c in range(NKH):
                nc.tensor.matmul(
                    qp_ps[:],
                    q_sb[:, c * B_CORE:(c + 1) * B_CORE],
                    wa_sb[:, c * H:(c + 1) * H],
                    start=(c == 0), stop=False,
                )
            nc.tensor.matmul(qp_ps[:], ones4, qb_sb, start=False, stop=True)
            # copy to SBUF row form, then transpose to columns [128, 4*NH]
            qp_row = st.tile([4, H], F32, tag="qprow")
            nc.scalar.copy(qp_row[:], qp_ps[:])
            qc_ps = sm_ps.tile([128, 4 * NH], F32, tag="sm")
            for hc in range(NH):
                nc.tensor.transpose(
                    qc_ps[:, hc * 4:(hc + 1) * 4],
                    qp_row[:, hc * 128:(hc + 1) * 128],
                    id4[:],
                )
            qcols = st.tile([128, 4 * NH], F32, tag="qcols")
            nc.scalar.copy(qcols[:], qc_ps[:])

            # ---------------- per batch ----------------
            for b in range(B_CORE):
                # -- keysT via DMA xbar transpose: 8 chunks [128, S] bf16
                kTt = kT_pool.tile([128, NKH * S], BF, tag="kT")
                for q in range(NSB):
                    nc.sync.dma_start(
                        kTt[:].rearrange("p (c s) -> p c s", c=NKH)[:, :, q * 512:(q + 1) * 512],
                        keysT_bf[b].rearrange("(c p) s -> p c s", p=128)[:, :, q * 512:(q + 1) * 512],
                    )

                exp_row = row_pool.tile([1, S], F32, tag="exp")
                den4 = small_pool.tile([1, 4], F32, tag="den")

                # -- kproj + tanh + scores + exp, streamed per s block.
                # The scores matmul trails its kproj group by one group so the
                # ScalarE tanh never stalls the PE.
                pending = []

                def emit_pending():
                    psb, phc, pet, ptile = pending.pop(0)
                    nc.tensor.matmul(
                        ptile[:], va_sb[:, phc:phc + 1], pet[:],
                        start=(phc == 0), stop=(phc == NH - 1),
                    )
                    if phc == NH - 1:
                        nc.scalar.activation(
                            exp_row[0:1, psb * 512:(psb + 1) * 512], ptile[:], EXP,
                            bias=va_b_ap,
                            accum_out=den4[0:1, psb:psb + 1],
                        )

                for sb in range(NSB):
                    sc_tile = sc_ps.tile([1, 512], F32, tag="sc")
                    for hc in range(NH):
                        eps = pe_ps.tile([128, 512], F32, tag="pe")
                        for c in range(NKH):
                            nc.tensor.matmul(
                                eps[:],
                                ua_sb[:, c * H + hc * 128: c * H + (hc + 1) * 128],
                                kTt[:, c * S + sb * 512: c * S + (sb + 1) * 512],
                                start=(c == 0), stop=(c == NKH - 1),
                            )
                        et = e_pool.tile([128, 512], F32R, tag="e")
                        tanh_ins = nc.scalar.activation(
                            et[:], eps[:], TANH,
                            bias=qcols[:, hc * 4 + b: hc * 4 + b + 1],
                        )
                        if sb == 0 and hc == 0:
                            first_tanh = tanh_ins
                        pending.append((sb, hc, et, sc_tile))
                        if len(pending) > 1:
                            emit_pending()
                # natural-layout keys for the context matmul: one fused DMA
                # (issued here so it doesn't compete with the kproj key loads)
                knt = knat_pool.tile([128, NSC * KH], BF, tag="knat")
                knat_dma = nc.gpsimd.dma_start(
                    knt[:].rearrange("p (sc j) -> p sc j", sc=NSC),
                    keys_bf[b].rearrange("(sc p) j -> p sc j", p=128),
                )
                add_dep_helper(knat_dma.ins, first_tanh.ins, sync=True,
                               reason="delay knat prefetch past batch start")

                while pending:
                    emit_pending()

                # -- denominator; normalize the exp row in place
                den1 = small_pool.tile([1, 1], F32, tag="den1")
                nc.vector.reduce_sum(den1[:], den4[:], axis=mybir.AxisListType.X)
                rcp = small_pool.tile([1, 1], F32, tag="rcp")
                nc.vector.reciprocal(rcp[:], den1[:])
                nc.vector.tensor_scalar_mul(exp_row[:], exp_row[:], rcp[:])
                nc.gpsimd.dma_start(w_out[b:b + 1, :], exp_row[:])

                # -- exp weight columns [128, 16] via PE transposes (unnormalized)
                wc_ps = sm_ps.tile([128, NSC], F32, tag="sm")
                for sc in range(NSC):
                    nc.tensor.transpose(
                        wc_ps[:, sc:sc + 1],
                        exp_row[0:1, sc * 128:(sc + 1) * 128],
                        one1[:],
                    )
                wcols = small_pool.tile([128, NSC], BF, tag="wcols")
                nc.scalar.copy(wcols[:], wc_ps[:])

                # -- context: two [1,512] psum halves over 16 s chunks
                ctx_ps = [sm_ps.tile([1, 512], F32, tag="sm", name=f"ctx_ps{i}")
                          for i in range(2)]
                for sc in range(NSC):
                    for half in range(2):
                        nc.tensor.matmul(
                            ctx_ps[half][:],
                            wcols[:, sc:sc + 1],
                            knt[:, sc * KH + half * 512: sc * KH + (half + 1) * 512],
                            start=(sc == 0), stop=(sc == NSC - 1),
                        )
                ctx_row = row_pool.tile([1, KH], F32, tag="ctxrow")
                for half in range(2):
                    nc.scalar.activation(
                        ctx_row[0:1, half * 512:(half + 1) * 512],
                        ctx_ps[half][:], COPY,
                    )

                if b == 0:
                    nc.gpsimd.dma_start(
                        ya_sb[:].rearrange("p (c h) -> p c h", c=NKH),
                        YaT.rearrange("(c p) h -> p c h", p=128),
                    )
                    yab_sb = st.tile([1, H], F32R, tag="yab")
                    nc.gpsimd.dma_start(yab_sb[:], yab_row[:])

                # -- context columns [128, 8] via PE transposes
                cT_ps = sm_ps.tile([128, NKH], F32, tag="sm")
                for c in range(NKH):
                    nc.tensor.transpose(
                        cT_ps[:, c:c + 1],
                        ctx_row[0:1, c * 128:(c + 1) * 128],
                        one1[:],
                    )
                cT = small_pool.tile([128, NKH], F32R, tag="cT")
                nc.scalar.copy(cT[:], cT_ps[:])

                # -- Ya projection: [1, 512] = sum_kh cT[:,c].T @ YaT chunk + Ya_b
                y_ps = sm_ps.tile([1, H], F32, tag="sm")
                for c in range(NKH):
                    nc.tensor.matmul(
                        y_ps[:], cT[:, c:c + 1], ya_sb[:, c * H:(c + 1) * H],
                        start=(c == 0), stop=False,
                    )
                nc.tensor.matmul(y_ps[:], one1r, yab_sb[:], start=False, stop=True)
                y_row = small_pool.tile([1, H], F32, tag="yrow")
                nc.scalar.copy(y_row[:], y_ps[:])
                nc.gpsimd.dma_start(ctx_out[b:b + 1, :], y_row[:])

    nc.compile()
    return nc


def _get_nc():
    if "nc" not in _CACHE:
        _install_ntff_hook()
        _CACHE["nc"] = _build_nc()
    return _CACHE["nc"]


def _prep_inputs(query, keys, Wa_w, Wa_b, Ua_w, Ua_b, Va_w, Va_b, Ya_w, Ya_b):
    """Host-side layout-only transforms + per-core sharding."""
    query = np.asarray(query, np.float32).reshape(B, KH)
    keys = np.asarray(keys, np.float32)
    keys_bf = keys.astype(BF16)  # [B, S, KH]

    WaT = np.ascontiguousarray(np.asarray(Wa_w, np.float32).T)          # [KH, H]
    UaT_bf = np.ascontiguousarray(np.asarray(Ua_w, np.float32).T).astype(BF16)
    YaT = np.ascontiguousarray(np.asarray(Ya_w, np.float32).T)          # [KH, H]
    yab_row = np.asarray(Ya_b, np.float32).reshape(1, H)
    rows_pack = np.zeros((1, 1024), np.float32)
    rows_pack[0, :H] = (np.asarray(Wa_b, np.float32) + np.asarray(Ua_b, np.float32))
    rows_pack[0, H] = np.float32(np.asarray(Va_b).reshape(-1)[0])
    rows_pack[0, H + 1:H + 5] = 1.0
    va_cols = np.ascontiguousarray(
        np.asarray(Va_w, np.float32).reshape(4, 128).T)                 # [128, 4]

    in_maps = []
    for i in range(N_CORES):
        bs = slice(i * B_CORE, (i + 1) * B_CORE)
        # smalls[:, 0:32]: q columns (q[p, c*4+b] = query[b, c*128+p]);
        # smalls[:, 32:36]: Va columns; rest zero-padded for fast DMA rows.
        smalls = np.zeros((128, 512), np.float32)
        smalls[:, 0:8 * B_CORE] = (
            query[bs].reshape(B_CORE, 8, 128).transpose(2, 1, 0).reshape(128, 8 * B_CORE)
        )
        smalls[:, 32:36] = va_cols
        in_maps.append({
            "keys_bf16": np.ascontiguousarray(keys_bf[bs]),
            "keysT_bf16": np.ascontiguousarray(keys_bf[bs].transpose(0, 2, 1)),
            "smalls": smalls,
            "WaT": WaT,
            "UaT_bf16": UaT_bf,
            "YaT": YaT,
            "rows_pack": rows_pack,
            "yab_row": yab_row,
        })
    return in_maps


def kernel(query, keys, Wa_w, Wa_b, Ua_w, Ua_b, Va_w, Va_b, Ya_w, Ya_b,
           _trace=False):
    from concourse.bass_utils import run_bass_kernel_spmd

    nc = _get_nc()
    in_maps = _prep_inputs(query, keys, Wa_w, Wa_b, Ua_w, Ua_b,
                           Va_w, Va_b, Ya_w, Ya_b)
    res = run_bass_kernel_spmd(
        nc, in_maps, core_ids=list(range(N_CORES)), trace=_trace
    )
    context = np.concatenate(
        [r["ctx_out"].reshape(B_CORE, 1, H) for r in res.results], axis=0
    ).astype(np.float32)
    weights = np.concatenate(
        [r["w_out"].reshape(B_CORE, 1, S) for r in res.results], axis=0
    ).astype(np.float32)
    if _trace:
        kernel._last_result = res
    return context, weights
